# revision 1
# baseline (speedup 1.0000x reference)
import os

# fp32-strict compile: the network has a tanh(low*(...)-high) stage with
# low ~ 1e4, which amplifies any bf16 matmul rounding upstream of it into
# O(1) output errors. Disable the compiler's default matmult auto-cast.
_flags = os.environ.get("NEURON_CC_FLAGS", "")
if "--auto-cast" not in _flags:
    os.environ["NEURON_CC_FLAGS"] = (_flags + " --auto-cast=none").strip()

import numpy as np
import jax
import jax.numpy as jnp

N_CORES = 8
_B = 256  # full batch; sharded N_CORES-way on the batch dim (pure data parallel)


def _conv(x, w, b):
    # torch Conv2d stride=2, padding=1, kernel=3; w: [out,in,3,3]
    y = jax.lax.conv_general_dilated(
        x, w, (2, 2), ((1, 1), (1, 1)),
        dimension_numbers=("NCHW", "OIHW", "NCHW"),
    )
    return y + b[None, :, None, None]


def _deconv(x, w, b):
    # torch ConvTranspose2d stride=2, padding=1, output_padding=1, kernel=3
    wt = jnp.flip(w, (2, 3)).transpose(1, 0, 2, 3)
    y = jax.lax.conv_general_dilated(
        x, wt, (1, 1), ((1, 2), (1, 2)),
        lhs_dilation=(2, 2),
        dimension_numbers=("NCHW", "OIHW", "NCHW"),
    )
    return y + b[None, :, None, None]


def _forward(x, p):
    relu = jax.nn.relu
    lrelu = lambda t: jax.nn.leaky_relu(t, 0.01)
    h = relu(_conv(x, p["conv1_w"], p["conv1_b"]))
    h = relu(_conv(h, p["conv2_w"], p["conv2_b"]))
    h = relu(_conv(h, p["conv3_w"], p["conv3_b"]))
    h = relu(_conv(h, p["conv4_w"], p["conv4_b"]))
    B = h.shape[0]
    h = h.reshape(B, -1)
    h = relu(h @ p["l2_w"].T + p["l2_b"])
    lin = h @ p["cl_w"].T + p["cl_b"]
    neur = jnp.tanh(jnp.tanh(p["low"] * (h @ p["n_w"].T + p["n_b"]) - p["high"]))
    h = relu(lin + neur)
    h = relu(h @ p["l4_w"].T + p["l4_b"])
    h = lrelu(h @ p["lL_w"].T + p["lL_b"])
    h = lrelu(h @ p["fc4_w"].T + p["fc4_b"])
    h = relu(h @ p["fc5_w"].T + p["fc5_b"])
    h = h.reshape(B, 8, 8, 8)
    h = _deconv(h, p["dc1_w"], p["dc1_b"])
    h = _deconv(h, p["dc2_w"], p["dc2_b"])
    h = _deconv(h, p["dc3_w"], p["dc3_b"])
    h = _deconv(h, p["dc4_w"], p["dc4_b"])
    return h


_fwd_pmapped = None


def kernel(**inputs):
    global _fwd_pmapped
    x = np.asarray(inputs["x"], dtype=np.float32)
    params = {
        k: np.asarray(v, dtype=np.float32) for k, v in inputs.items() if k != "x"
    }
    devs = jax.devices()[:N_CORES]
    if _fwd_pmapped is None:
        _fwd_pmapped = jax.pmap(
            _forward, axis_name="i", in_axes=(0, None), devices=devs
        )
    b = x.shape[0]
    assert b % N_CORES == 0, f"batch {b} not divisible by {N_CORES}"
    xs = x.reshape(N_CORES, b // N_CORES, *x.shape[1:])
    out = _fwd_pmapped(xs, params)
    out = np.asarray(out, dtype=np.float32).reshape(b, 3, 128, 128)
    return out

